# revision 21
# baseline (speedup 1.0000x reference)
"""AttnBlock (GroupNorm -> QKV 1x1 conv -> spatial attention with softmax over
query-H axis -> output projection + residual) for B=8, C=128, H=W=48 on 8
Trainium2 NeuronCores, data-parallel over batch (1 batch per core).

Math per batch (N = H*W = 2304 spatial positions, C = 128 channels):
  xn = GroupNorm(x; 32 groups of 4 channels)
  q/k/v = W @ xn + b              (per-position 1x1 conv = C x C matmul)
  S[q', kp] = q[:,q'] . k[:,kp] / sqrt(C)
  attn = softmax over the query-H axis: for fixed (w, kp), normalize over h
  ctx[c, (h,w)] = sum_kp attn[(h,w), kp] * v[c, kp]
  out = x + Wo @ ctx + bo

Device mapping:
  - Channels on the 128 SBUF partitions; spatial positions on the free axis.
  - S computed transposed (S^T [kp, q']) per 128-key chunk so the softmax
    reduction (over h) runs along the free axis (VectorE grouped reduce).
  - Queries stored w-major (q' = w*48 + h) so each softmax group of 48 h
    values is contiguous; the reorder is free (permuted APs on the projection
    evacuations).
  - Projections in float32r; q/k/v rounded to bf16 once at evacuation, so the
    attention matmuls run at full bf16 rate with fast weight loads.
  - All 18 normalized-E (bf16) chunk tiles stay resident in SBUF; the S^T
    staging PSUM pool gets two slots so TensorE/ScalarE ping-pong freely.
  - The GroupNorm affine is folded into the projection weights (no separate
    normalize pass over x).
  - ctx accumulates in 4 PSUM banks for query columns 0:2048 (interleaved,
    lagged four chunks behind the softmax chain); the 256-column tail gets a
    short dense pass at the end.
  - Normalize-muls split between GpSimd (13 chunks) and VectorE (5 chunks);
    the residual add runs per output block on VectorE.
"""

import sys

sys.path.insert(0, "/opt/trn_rl_repo")

import numpy as np

import concourse.bass as bass
import concourse.mybir as mybir
import concourse.tile as tile
from concourse import bacc, bass_utils

B, C, H, W = 8, 128, 48, 48
N = H * W  # 2304
GROUPS = 32
GSIZE = C // GROUPS
EPS = 1e-5
NCORES = 8

F32 = mybir.dt.float32
F32R = mybir.dt.float32r
BF16 = mybir.dt.bfloat16
AF = mybir.ActivationFunctionType
OP = mybir.AluOpType

NCHUNK = N // 128  # 18 key chunks
QG = 768  # S^T staging / exp granularity
NQG = N // QG  # 3
CTX_LIVE = [0, 512, 1024, 1536]  # 4 psum-resident ctx banks (512 wide each)
TAIL_OFF, TAIL_SZ = 2048, 256  # final ctx region, computed in a tail pass
DVE_MUL_CHUNKS = {0, 4, 8, 12, 16}  # normalize-mul on VectorE; rest on GpSimd
GP_HALVE_CHUNKS = {2, 7, 11, 14}  # GpSimd pre-halves E over h before the DVE reduce


def _build_program():
    nc = bacc.Bacc("TRN2", target_bir_lowering=False, debug=False)

    def din(name, shape, dt=F32):
        return nc.dram_tensor(name, shape, dt, kind="ExternalInput")

    x_d = din("x", [C, N], F32R)
    gnw_d = din("gn_w", [C, 1])
    gnb_d = din("gn_b", [C, 1])
    wqT_d = din("wqT", [C, C], F32R)
    wkT_d = din("wkT", [C, C], F32R)
    wvT_d = din("wvT", [C, C], F32R)
    woT_d = din("woT", [C, C], F32R)
    bq_d = din("bq", [C, 1])
    bk_d = din("bk", [C, 1])
    bv_d = din("bv", [C, 1])
    bo_d = din("bo", [C, 1])
    gmat_d = din("gmat", [C, GROUPS], F32R)
    gexp_d = din("gexp", [GROUPS, C], F32R)
    ident_d = din("ident", [C, C], BF16)
    out_d = nc.dram_tensor("out", [C, N], F32, kind="ExternalOutput")

    with tile.TileContext(nc) as tc:
        with (
            tc.tile_pool(name="const", bufs=1) as const,
            tc.tile_pool(name="data", bufs=1) as data,
            tc.tile_pool(name="small", bufs=1) as small,
            tc.tile_pool(name="soft", bufs=6) as soft,
            tc.tile_pool(name="epool", bufs=NCHUNK) as epool,
        ):
            # ---- input loads (x first: GroupNorm depends only on it) ----
            tx = data.tile([C, N], F32R)
            nc.sync.dma_start(tx[:], x_d[:])
            txf = tx[:].bitcast(F32)

            wqT = const.tile([C, C], F32R)
            wkT = const.tile([C, C], F32R)
            wvT = const.tile([C, C], F32R)
            woT = const.tile([C, C], F32R)
            gmat = const.tile([C, GROUPS], F32R)
            gexp = const.tile([GROUPS, C], F32R)
            ident = const.tile([C, C], BF16)
            gnw = const.tile([C, 1], F32)
            gnb = const.tile([C, 1], F32)
            bq = const.tile([C, 1], F32)
            bk = const.tile([C, 1], F32)
            bv = const.tile([C, 1], F32)
            bo = const.tile([C, 1], F32)
            for t, d in [
                (gmat, gmat_d), (gexp, gexp_d), (gnw, gnw_d), (gnb, gnb_d),
                (wqT, wqT_d), (wkT, wkT_d), (wvT, wvT_d), (woT, woT_d),
                (ident, ident_d),
                (bq, bq_d), (bk, bk_d), (bv, bv_d), (bo, bo_d),
            ]:
                nc.sync.dma_start(t[:], d[:])

            # ---- GroupNorm statistics ----
            sq_scratch = data.tile([C, N], F32)
            stats_f = small.tile([C, 2], F32)
            nc.vector.tensor_reduce(
                stats_f[:, 0:1], txf, axis=mybir.AxisListType.X, op=OP.add
            )
            nc.scalar.activation(
                sq_scratch[:], txf, AF.Square, accum_out=stats_f[:, 1:2]
            )
            stats = small.tile([C, 2], F32R)
            nc.vector.tensor_copy(stats[:], stats_f[:])

            with tc.tile_pool(name="gnps", bufs=1, space="PSUM") as gnps:
                psg = gnps.tile([GROUPS, 2], F32)
                nc.tensor.matmul(psg[:], gmat[:], stats[:], start=True, stop=True)

                inv_n = 1.0 / (GSIZE * N)
                t32 = small.tile([GROUPS, 4], F32)
                nc.vector.tensor_scalar_mul(t32[:, 0:1], psg[:, 0:1], inv_n)
                nc.vector.tensor_scalar_mul(t32[:, 1:2], psg[:, 1:2], inv_n)
                nc.vector.tensor_mul(t32[:, 2:3], t32[:, 0:1], t32[:, 0:1])
                nc.vector.tensor_sub(t32[:, 3:4], t32[:, 1:2], t32[:, 2:3])
                eps_t = small.tile([GROUPS, 1], F32)
                nc.vector.memset(eps_t[:], EPS)
                nc.scalar.activation(t32[:, 2:3], t32[:, 3:4], AF.Ln, bias=eps_t[:])
                rstd_f = small.tile([GROUPS, 1], F32)
                nc.scalar.activation(rstd_f[:], t32[:, 2:3], AF.Exp, scale=-0.5)
                mstat = small.tile([GROUPS, 2], F32R)
                nc.vector.tensor_copy(mstat[:, 0:1], t32[:, 0:1])
                nc.vector.tensor_copy(mstat[:, 1:2], rstd_f[:])

                pse = gnps.tile([C, 2], F32)
                nc.tensor.matmul(pse[:], gexp[:], mstat[:], start=True, stop=True)

                A_sb = small.tile([C, 1], F32)
                B_sb = small.tile([C, 1], F32)
                nc.vector.tensor_mul(A_sb[:], pse[:, 1:2], gnw[:])
                nc.vector.tensor_mul(B_sb[:], pse[:, 0:1], A_sb[:])
                nc.vector.tensor_sub(B_sb[:], gnb[:], B_sb[:])

            # ---- fold the GroupNorm affine into the projection weights:
            # ---- q = Wq(A*x + B) + bq = (Wq diag(A)) x + (Wq B + bq)
            wq2 = small.tile([C, C], F32R)
            wk2 = small.tile([C, C], F32R)
            wv2 = small.tile([C, C], F32R)
            bq2 = small.tile([C, 1], F32)
            bk2 = small.tile([C, 1], F32)
            bv2 = small.tile([C, 1], F32)
            with tc.tile_pool(name="foldps", bufs=1, space="PSUM") as foldps:
                psb = foldps.tile([C, 4], F32)
                for i, (wT, w2, bias, b2) in enumerate((
                    (wqT, wq2, bq, bq2),
                    (wkT, wk2, bk, bk2),
                    (wvT, wv2, bv, bv2),
                )):
                    nc.vector.tensor_scalar_mul(w2[:], wT[:], A_sb[:])
                    nc.tensor.matmul(
                        psb[:, i : i + 1], wT[:].bitcast(F32), B_sb[:],
                        start=True, stop=True,
                    )
                    nc.vector.tensor_add(b2[:], psb[:, i : i + 1], bias[:])

            # ---- Q/K/V projections (q written w-major); q,k first so the
            # ---- attention loop can start while v/vT still evacuates ----
            q = data.tile([C, N], BF16)
            k = data.tile([C, N], BF16)
            v = data.tile([C, N], BF16)
            q_wh = q[:].rearrange("p (w h) -> p h w", h=H)
            with tc.tile_pool(name="projps", bufs=2, space="PSUM") as projps:
                for wT, bias, dst, permute in (
                    (wq2, bq2, q, True),
                    (wk2, bk2, k, False),
                    (wv2, bv2, v, False),
                ):
                    for g in range(NQG):
                        pp = projps.tile([C, QG], F32, tag="pp")
                        o = g * QG
                        nc.tensor.matmul(
                            pp[:, 0:512], wT[:], tx[:, o : o + 512],
                            start=True, stop=True,
                        )
                        nc.tensor.matmul(
                            pp[:, 512:QG], wT[:], tx[:, o + 512 : o + QG],
                            start=True, stop=True,
                        )
                        if permute:
                            outv = q_wh[:, 16 * g : 16 * (g + 1), :]
                            inv = pp[:].rearrange("p (h w) -> p h w", w=W)
                        else:
                            outv = dst[:, o : o + QG]
                            inv = pp[:, :]
                        nc.scalar.activation(outv, inv, AF.Identity, bias=bias[:])

                vT = data.tile([C, NCHUNK * C], BF16)
                for grp in range(0, NCHUNK, 4):
                    cnt = min(4, NCHUNK - grp)
                    pvt = projps.tile([C, 512], BF16, tag="pvt")
                    for j in range(cnt):
                        ch = grp + j
                        nc.tensor.transpose(
                            pvt[:, 128 * j : 128 * (j + 1)],
                            v[:, 128 * ch : 128 * (ch + 1)],
                            ident[:],
                        )
                    nc.vector.tensor_copy(
                        vT[:, 128 * grp : 128 * (grp + cnt)], pvt[:, : 128 * cnt]
                    )

            # ---- main attention loop ----
            e_tiles = [None] * NCHUNK
            ctx_all = data.tile([C, N], F32R)
            with (
                tc.tile_pool(name="ctxps", bufs=1, space="PSUM") as ctxps,
                tc.tile_pool(name="sps", bufs=2, space="PSUM") as sps,
            ):
                ctx_ps = [
                    ctxps.tile([C, 512], F32, tag=f"ctx{i}", name=f"ctx_ps{i}")
                    for i in range(len(CTX_LIVE))
                ]

                def emit_av(ch, part):
                    ec = e_tiles[ch]
                    for i in ([0, 1], [2], [3])[part]:
                        o = CTX_LIVE[i]
                        nc.tensor.matmul(
                            ctx_ps[i][:, :],
                            vT[:, 128 * ch : 128 * (ch + 1)],
                            ec[:, o : o + 512],
                            start=(ch == 0),
                            stop=(ch == NCHUNK - 1),
                        )

                for it in range(NCHUNK + 4):
                    ch = it if it < NCHUNK else None
                    av = it - 4
                    if ch is not None:
                        ec = epool.tile([C, N], BF16, tag="E", name=f"E_{ch}")
                        e_tiles[ch] = ec
                        klhs = k[:, 128 * ch : 128 * (ch + 1)]
                        for g in range(NQG):
                            ps = sps.tile([C, QG], F32, tag="spsum")
                            o = g * QG
                            nc.tensor.matmul(
                                ps[:, 0:512], klhs, q[:, o : o + 512],
                                start=True, stop=True,
                            )
                            nc.tensor.matmul(
                                ps[:, 512:QG], klhs, q[:, o + 512 : o + QG],
                                start=True, stop=True,
                            )
                            nc.scalar.activation(ec[:, o : o + QG], ps[:, :], AF.Exp)
                            if av >= 0:
                                emit_av(av, g)
                    else:
                        for g in range(NQG):
                            emit_av(av, g)

                    if ch is None:
                        continue
                    dsum = soft.tile([C, W], F32, tag="D")
                    ev3 = ec[:].rearrange("p (w h) -> p w h", h=H)
                    if ch in GP_HALVE_CHUNKS:
                        th = soft.tile([C, W * (H // 2)], BF16, tag="T")
                        nc.gpsimd.tensor_tensor(
                            out=th[:].rearrange("p (w h) -> p w h", h=H // 2),
                            in0=ev3[:, :, 0 : H // 2],
                            in1=ev3[:, :, H // 2 : H],
                            op=OP.add,
                        )
                        nc.vector.tensor_reduce(
                            dsum[:],
                            th[:].rearrange("p (w h) -> p w h", h=H // 2),
                            axis=mybir.AxisListType.X,
                            op=OP.add,
                        )
                    else:
                        nc.vector.tensor_reduce(
                            dsum[:], ev3, axis=mybir.AxisListType.X, op=OP.add
                        )
                    rden = soft.tile([C, W], F32, tag="R")
                    nc.vector.reciprocal_approx_fast(rden[:], dsum[:])
                    ev = ec[:].rearrange("p (w h) -> p w h", h=H)
                    if ch in DVE_MUL_CHUNKS:
                        nc.vector.tensor_tensor(
                            out=ev, in0=ev,
                            in1=rden[:, :, None].to_broadcast([C, W, H]),
                            op=OP.mult,
                        )
                    else:
                        rden_b = soft.tile([C, W], BF16, tag="Rb")
                        nc.vector.tensor_copy(rden_b[:], rden[:])
                        nc.gpsimd.tensor_tensor(
                            out=ev, in0=ev,
                            in1=rden_b[:, :, None].to_broadcast([C, W, H]),
                            op=OP.mult,
                        )

                for i, o in enumerate(CTX_LIVE):
                    nc.scalar.copy(ctx_all[:, o : o + 512], ctx_ps[i][:, :])

            # ---- ctx tail (columns 2048:2304) + output projection + residual ----
            out_nat = data.tile([C, N], F32)
            out_wh = out_nat[:].rearrange("p (h w) -> p w h", w=W)
            with tc.tile_pool(name="ops", bufs=2, space="PSUM") as ops:
                tail = ops.tile([C, TAIL_SZ], F32, tag="tail")
                for ch in range(NCHUNK):
                    nc.tensor.matmul(
                        tail[:, :],
                        vT[:, 128 * ch : 128 * (ch + 1)],
                        e_tiles[ch][:, TAIL_OFF : TAIL_OFF + TAIL_SZ],
                        start=(ch == 0),
                        stop=(ch == NCHUNK - 1),
                    )
                nc.scalar.copy(ctx_all[:, TAIL_OFF : TAIL_OFF + TAIL_SZ], tail[:, :])

                for g in range(NQG):
                    po = ops.tile([C, QG], F32, tag="po")
                    o = g * QG
                    nc.tensor.matmul(
                        po[:, 0:512], woT[:], ctx_all[:, o : o + 512],
                        start=True, stop=True,
                    )
                    nc.tensor.matmul(
                        po[:, 512:QG], woT[:], ctx_all[:, o + 512 : o + QG],
                        start=True, stop=True,
                    )
                    ov = out_wh[:, 16 * g : 16 * (g + 1), :]
                    nc.scalar.activation(
                        ov,
                        po[:].rearrange("p (w h) -> p w h", h=H),
                        AF.Identity,
                        bias=bo[:],
                    )
                    txv = txf.rearrange("p (h w) -> p w h", w=W)
                    nc.vector.tensor_tensor(
                        out=ov, in0=ov,
                        in1=txv[:, 16 * g : 16 * (g + 1), :],
                        op=OP.add,
                    )
            nc.sync.dma_start(out_d[:], out_nat[:])

    nc.compile()
    return nc


_PROGRAM_CACHE = None


def kernel(**inputs: np.ndarray) -> np.ndarray:
    global _PROGRAM_CACHE
    if _PROGRAM_CACHE is None:
        _PROGRAM_CACHE = _build_program()
    nc = _PROGRAM_CACHE

    import ml_dtypes

    f32 = lambda a: np.ascontiguousarray(np.asarray(a), dtype=np.float32)
    x = f32(inputs["x"])
    scale = 1.0 / np.sqrt(np.float32(C))

    gmat = np.zeros((C, GROUPS), np.float32)
    gmat[np.arange(C), np.arange(C) // GSIZE] = 1.0

    shared = {
        "gn_w": f32(inputs["gn_w"]).reshape(C, 1),
        "gn_b": f32(inputs["gn_b"]).reshape(C, 1),
        "wqT": np.ascontiguousarray(f32(inputs["wq"]).T * scale),
        "wkT": np.ascontiguousarray(f32(inputs["wk"]).T),
        "wvT": np.ascontiguousarray(f32(inputs["wv"]).T),
        "woT": np.ascontiguousarray(f32(inputs["wo"]).T),
        "bq": f32(inputs["bq"]).reshape(C, 1) * scale,
        "bk": f32(inputs["bk"]).reshape(C, 1),
        "bv": f32(inputs["bv"]).reshape(C, 1),
        "bo": f32(inputs["bo"]).reshape(C, 1),
        "gmat": gmat,
        "gexp": np.ascontiguousarray(gmat.T),
        "ident": np.eye(C).astype(ml_dtypes.bfloat16),
    }
    in_maps = [
        {**shared, "x": np.ascontiguousarray(x[b].reshape(C, N))} for b in range(B)
    ]

    res = bass_utils.run_bass_kernel_spmd(nc, in_maps, core_ids=list(range(NCORES)))
    out = np.stack([res.results[b]["out"].reshape(C, H, W) for b in range(B)])
    return out.astype(np.float32)


# revision 22
# speedup vs baseline: 1.0337x; 1.0337x over previous
"""AttnBlock (GroupNorm -> QKV 1x1 conv -> spatial attention with softmax over
query-H axis -> output projection + residual) for B=8, C=128, H=W=48 on 8
Trainium2 NeuronCores, data-parallel over batch (1 batch per core).

Math per batch (N = H*W = 2304 spatial positions, C = 128 channels):
  xn = GroupNorm(x; 32 groups of 4 channels)
  q/k/v = W @ xn + b              (per-position 1x1 conv = C x C matmul)
  S[q', kp] = q[:,q'] . k[:,kp] / sqrt(C)
  attn = softmax over the query-H axis: for fixed (w, kp), normalize over h
  ctx[c, (h,w)] = sum_kp attn[(h,w), kp] * v[c, kp]
  out = x + Wo @ ctx + bo

Device mapping:
  - Channels on the 128 SBUF partitions; spatial positions on the free axis.
  - S computed transposed (S^T [kp, q']) per 128-key chunk so the softmax
    reduction (over h) runs along the free axis (VectorE grouped reduce).
  - Queries stored w-major (q' = w*48 + h) so each softmax group of 48 h
    values is contiguous; the reorder is free (permuted APs on the projection
    evacuations).
  - Projections in float32r; q/k/v rounded to bf16 once at evacuation, so the
    attention matmuls run at full bf16 rate with fast weight loads.
  - All 18 normalized-E (bf16) chunk tiles stay resident in SBUF; the S^T
    staging PSUM pool gets two slots so TensorE/ScalarE ping-pong freely.
  - The GroupNorm affine is folded into the projection weights (no separate
    normalize pass over x).
  - ctx accumulates in 4 PSUM banks for query columns 0:2048 (interleaved,
    lagged four chunks behind the softmax chain); the 256-column tail gets a
    short dense pass at the end.
  - Normalize-muls split between GpSimd (13 chunks) and VectorE (5 chunks);
    the residual add runs per output block on VectorE.
"""

import sys

sys.path.insert(0, "/opt/trn_rl_repo")

import numpy as np

import concourse.bass as bass
import concourse.mybir as mybir
import concourse.tile as tile
from concourse import bacc, bass_utils

B, C, H, W = 8, 128, 48, 48
N = H * W  # 2304
GROUPS = 32
GSIZE = C // GROUPS
EPS = 1e-5
NCORES = 8

F32 = mybir.dt.float32
F32R = mybir.dt.float32r
BF16 = mybir.dt.bfloat16
AF = mybir.ActivationFunctionType
OP = mybir.AluOpType

NCHUNK = N // 128  # 18 key chunks
QG = 768  # S^T staging / exp granularity
NQG = N // QG  # 3
CTX_LIVE = [0, 512, 1024, 1536]  # 4 psum-resident ctx banks (512 wide each)
TAIL_OFF, TAIL_SZ = 2048, 256  # final ctx region, computed in a tail pass
DVE_MUL_CHUNKS = {0, 4, 8, 12, 16}  # normalize-mul on VectorE; rest on GpSimd


def _build_program():
    nc = bacc.Bacc("TRN2", target_bir_lowering=False, debug=False)

    def din(name, shape, dt=F32):
        return nc.dram_tensor(name, shape, dt, kind="ExternalInput")

    x_d = din("x", [C, N], F32R)
    gnw_d = din("gn_w", [C, 1])
    gnb_d = din("gn_b", [C, 1])
    wqT_d = din("wqT", [C, C], F32R)
    wkT_d = din("wkT", [C, C], F32R)
    wvT_d = din("wvT", [C, C], F32R)
    woT_d = din("woT", [C, C], F32R)
    bq_d = din("bq", [C, 1])
    bk_d = din("bk", [C, 1])
    bv_d = din("bv", [C, 1])
    bo_d = din("bo", [C, 1])
    gmat_d = din("gmat", [C, GROUPS], F32R)
    gexp_d = din("gexp", [GROUPS, C], F32R)
    ident_d = din("ident", [C, C], BF16)
    out_d = nc.dram_tensor("out", [C, N], F32, kind="ExternalOutput")

    with tile.TileContext(nc) as tc:
        with (
            tc.tile_pool(name="const", bufs=1) as const,
            tc.tile_pool(name="data", bufs=1) as data,
            tc.tile_pool(name="small", bufs=1) as small,
            tc.tile_pool(name="soft", bufs=6) as soft,
            tc.tile_pool(name="epool", bufs=NCHUNK) as epool,
        ):
            # ---- input loads (x first: GroupNorm depends only on it) ----
            tx = data.tile([C, N], F32R)
            nc.sync.dma_start(tx[:], x_d[:])
            txf = tx[:].bitcast(F32)

            wqT = const.tile([C, C], F32R)
            wkT = const.tile([C, C], F32R)
            wvT = const.tile([C, C], F32R)
            woT = const.tile([C, C], F32R)
            gmat = const.tile([C, GROUPS], F32R)
            gexp = const.tile([GROUPS, C], F32R)
            ident = const.tile([C, C], BF16)
            gnw = const.tile([C, 1], F32)
            gnb = const.tile([C, 1], F32)
            bq = const.tile([C, 1], F32)
            bk = const.tile([C, 1], F32)
            bv = const.tile([C, 1], F32)
            bo = const.tile([C, 1], F32)
            for t, d in [
                (gmat, gmat_d), (gexp, gexp_d), (gnw, gnw_d), (gnb, gnb_d),
                (wqT, wqT_d), (wkT, wkT_d), (wvT, wvT_d), (woT, woT_d),
                (ident, ident_d),
                (bq, bq_d), (bk, bk_d), (bv, bv_d), (bo, bo_d),
            ]:
                nc.sync.dma_start(t[:], d[:])

            # ---- GroupNorm statistics ----
            sq_scratch = data.tile([C, N], F32)
            stats_f = small.tile([C, 2], F32)
            nc.vector.tensor_reduce(
                stats_f[:, 0:1], txf, axis=mybir.AxisListType.X, op=OP.add
            )
            nc.scalar.activation(
                sq_scratch[:], txf, AF.Square, accum_out=stats_f[:, 1:2]
            )
            stats = small.tile([C, 2], F32R)
            nc.vector.tensor_copy(stats[:], stats_f[:])

            with tc.tile_pool(name="gnps", bufs=1, space="PSUM") as gnps:
                psg = gnps.tile([GROUPS, 2], F32)
                nc.tensor.matmul(psg[:], gmat[:], stats[:], start=True, stop=True)

                inv_n = 1.0 / (GSIZE * N)
                t32 = small.tile([GROUPS, 4], F32)
                nc.vector.tensor_scalar_mul(t32[:, 0:1], psg[:, 0:1], inv_n)
                nc.vector.tensor_scalar_mul(t32[:, 1:2], psg[:, 1:2], inv_n)
                nc.vector.tensor_mul(t32[:, 2:3], t32[:, 0:1], t32[:, 0:1])
                nc.vector.tensor_sub(t32[:, 3:4], t32[:, 1:2], t32[:, 2:3])
                eps_t = small.tile([GROUPS, 1], F32)
                nc.vector.memset(eps_t[:], EPS)
                nc.scalar.activation(t32[:, 2:3], t32[:, 3:4], AF.Ln, bias=eps_t[:])
                rstd_f = small.tile([GROUPS, 1], F32)
                nc.scalar.activation(rstd_f[:], t32[:, 2:3], AF.Exp, scale=-0.5)
                mstat = small.tile([GROUPS, 2], F32R)
                nc.vector.tensor_copy(mstat[:, 0:1], t32[:, 0:1])
                nc.vector.tensor_copy(mstat[:, 1:2], rstd_f[:])

                pse = gnps.tile([C, 2], F32)
                nc.tensor.matmul(pse[:], gexp[:], mstat[:], start=True, stop=True)

                A_sb = small.tile([C, 1], F32)
                B_sb = small.tile([C, 1], F32)
                nc.vector.tensor_mul(A_sb[:], pse[:, 1:2], gnw[:])
                nc.vector.tensor_mul(B_sb[:], pse[:, 0:1], A_sb[:])
                nc.vector.tensor_sub(B_sb[:], gnb[:], B_sb[:])

            # ---- fold the GroupNorm affine into the projection weights:
            # ---- q = Wq(A*x + B) + bq = (Wq diag(A)) x + (Wq B + bq)
            wq2 = small.tile([C, C], F32R)
            wk2 = small.tile([C, C], F32R)
            wv2 = small.tile([C, C], F32R)
            bq2 = small.tile([C, 1], F32)
            bk2 = small.tile([C, 1], F32)
            bv2 = small.tile([C, 1], F32)
            with tc.tile_pool(name="foldps", bufs=1, space="PSUM") as foldps:
                psb = foldps.tile([C, 4], F32)
                for i, (wT, w2, bias, b2) in enumerate((
                    (wqT, wq2, bq, bq2),
                    (wkT, wk2, bk, bk2),
                    (wvT, wv2, bv, bv2),
                )):
                    nc.vector.tensor_scalar_mul(w2[:], wT[:], A_sb[:])
                    nc.tensor.matmul(
                        psb[:, i : i + 1], wT[:].bitcast(F32), B_sb[:],
                        start=True, stop=True,
                    )
                    nc.vector.tensor_add(b2[:], psb[:, i : i + 1], bias[:])

            # ---- Q/K/V projections (q written w-major); q,k first so the
            # ---- attention loop can start while v/vT still evacuates ----
            q = data.tile([C, N], BF16)
            k = data.tile([C, N], BF16)
            v = data.tile([C, N], BF16)
            q_wh = q[:].rearrange("p (w h) -> p h w", h=H)
            with tc.tile_pool(name="projps", bufs=2, space="PSUM") as projps:
                for wT, bias, dst, permute in (
                    (wq2, bq2, q, True),
                    (wk2, bk2, k, False),
                    (wv2, bv2, v, False),
                ):
                    for g in range(NQG):
                        pp = projps.tile([C, QG], F32, tag="pp")
                        o = g * QG
                        nc.tensor.matmul(
                            pp[:, 0:512], wT[:], tx[:, o : o + 512],
                            start=True, stop=True,
                        )
                        nc.tensor.matmul(
                            pp[:, 512:QG], wT[:], tx[:, o + 512 : o + QG],
                            start=True, stop=True,
                        )
                        if permute:
                            outv = q_wh[:, 16 * g : 16 * (g + 1), :]
                            inv = pp[:].rearrange("p (h w) -> p h w", w=W)
                        else:
                            outv = dst[:, o : o + QG]
                            inv = pp[:, :]
                        nc.scalar.activation(outv, inv, AF.Identity, bias=bias[:])

                vT = data.tile([C, NCHUNK * C], BF16)
                for grp in range(0, NCHUNK, 4):
                    cnt = min(4, NCHUNK - grp)
                    pvt = projps.tile([C, 512], BF16, tag="pvt")
                    for j in range(cnt):
                        ch = grp + j
                        nc.tensor.transpose(
                            pvt[:, 128 * j : 128 * (j + 1)],
                            v[:, 128 * ch : 128 * (ch + 1)],
                            ident[:],
                        )
                    nc.vector.tensor_copy(
                        vT[:, 128 * grp : 128 * (grp + cnt)], pvt[:, : 128 * cnt]
                    )

            # ---- main attention loop ----
            e_tiles = [None] * NCHUNK
            ctx_all = data.tile([C, N], F32R)
            with (
                tc.tile_pool(name="ctxps", bufs=1, space="PSUM") as ctxps,
                tc.tile_pool(name="sps", bufs=2, space="PSUM") as sps,
            ):
                ctx_ps = [
                    ctxps.tile([C, 512], F32, tag=f"ctx{i}", name=f"ctx_ps{i}")
                    for i in range(len(CTX_LIVE))
                ]

                def emit_av(ch, part):
                    ec = e_tiles[ch]
                    for i in ([0, 1], [2], [3])[part]:
                        o = CTX_LIVE[i]
                        nc.tensor.matmul(
                            ctx_ps[i][:, :],
                            vT[:, 128 * ch : 128 * (ch + 1)],
                            ec[:, o : o + 512],
                            start=(ch == 0),
                            stop=(ch == NCHUNK - 1),
                        )

                for it in range(NCHUNK + 4):
                    ch = it if it < NCHUNK else None
                    av = it - 4
                    if ch is not None:
                        ec = epool.tile([C, N], BF16, tag="E", name=f"E_{ch}")
                        e_tiles[ch] = ec
                        klhs = k[:, 128 * ch : 128 * (ch + 1)]
                        for g in range(NQG):
                            ps = sps.tile([C, QG], F32, tag="spsum")
                            o = g * QG
                            nc.tensor.matmul(
                                ps[:, 0:512], klhs, q[:, o : o + 512],
                                start=True, stop=True,
                            )
                            nc.tensor.matmul(
                                ps[:, 512:QG], klhs, q[:, o + 512 : o + QG],
                                start=True, stop=True,
                            )
                            nc.scalar.activation(ec[:, o : o + QG], ps[:, :], AF.Exp)
                            if av >= 0:
                                emit_av(av, g)
                    else:
                        for g in range(NQG):
                            emit_av(av, g)

                    if ch is None:
                        continue
                    dsum = soft.tile([C, W], F32, tag="D")
                    nc.vector.tensor_reduce(
                        dsum[:],
                        ec[:].rearrange("p (w h) -> p w h", h=H),
                        axis=mybir.AxisListType.X,
                        op=OP.add,
                    )
                    rden = soft.tile([C, W], F32, tag="R")
                    nc.vector.reciprocal_approx_fast(rden[:], dsum[:])
                    ev = ec[:].rearrange("p (w h) -> p w h", h=H)
                    if ch in DVE_MUL_CHUNKS:
                        nc.vector.tensor_tensor(
                            out=ev, in0=ev,
                            in1=rden[:, :, None].to_broadcast([C, W, H]),
                            op=OP.mult,
                        )
                    else:
                        rden_b = soft.tile([C, W], BF16, tag="Rb")
                        nc.vector.tensor_copy(rden_b[:], rden[:])
                        nc.gpsimd.tensor_tensor(
                            out=ev, in0=ev,
                            in1=rden_b[:, :, None].to_broadcast([C, W, H]),
                            op=OP.mult,
                        )

                for i, o in enumerate(CTX_LIVE):
                    nc.scalar.copy(ctx_all[:, o : o + 512], ctx_ps[i][:, :])

            # ---- ctx tail (columns 2048:2304) + output projection + residual ----
            out_nat = data.tile([C, N], F32)
            out_wh = out_nat[:].rearrange("p (h w) -> p w h", w=W)
            with tc.tile_pool(name="ops", bufs=2, space="PSUM") as ops:
                tail = ops.tile([C, TAIL_SZ], F32, tag="tail")
                for ch in range(NCHUNK):
                    nc.tensor.matmul(
                        tail[:, :],
                        vT[:, 128 * ch : 128 * (ch + 1)],
                        e_tiles[ch][:, TAIL_OFF : TAIL_OFF + TAIL_SZ],
                        start=(ch == 0),
                        stop=(ch == NCHUNK - 1),
                    )
                nc.scalar.copy(ctx_all[:, TAIL_OFF : TAIL_OFF + TAIL_SZ], tail[:, :])

                for g in range(NQG):
                    po = ops.tile([C, QG], F32, tag="po")
                    o = g * QG
                    nc.tensor.matmul(
                        po[:, 0:512], woT[:], ctx_all[:, o : o + 512],
                        start=True, stop=True,
                    )
                    nc.tensor.matmul(
                        po[:, 512:QG], woT[:], ctx_all[:, o + 512 : o + QG],
                        start=True, stop=True,
                    )
                    ov = out_wh[:, 16 * g : 16 * (g + 1), :]
                    nc.scalar.activation(
                        ov,
                        po[:].rearrange("p (w h) -> p w h", h=H),
                        AF.Identity,
                        bias=bo[:],
                    )
                    txv = txf.rearrange("p (h w) -> p w h", w=W)
                    nc.vector.tensor_tensor(
                        out=ov, in0=ov,
                        in1=txv[:, 16 * g : 16 * (g + 1), :],
                        op=OP.add,
                    )
            nc.sync.dma_start(out_d[:], out_nat[:])

    nc.compile()
    return nc


_PROGRAM_CACHE = None


def kernel(**inputs: np.ndarray) -> np.ndarray:
    global _PROGRAM_CACHE
    if _PROGRAM_CACHE is None:
        _PROGRAM_CACHE = _build_program()
    nc = _PROGRAM_CACHE

    import ml_dtypes

    f32 = lambda a: np.ascontiguousarray(np.asarray(a), dtype=np.float32)
    x = f32(inputs["x"])
    scale = 1.0 / np.sqrt(np.float32(C))

    gmat = np.zeros((C, GROUPS), np.float32)
    gmat[np.arange(C), np.arange(C) // GSIZE] = 1.0

    shared = {
        "gn_w": f32(inputs["gn_w"]).reshape(C, 1),
        "gn_b": f32(inputs["gn_b"]).reshape(C, 1),
        "wqT": np.ascontiguousarray(f32(inputs["wq"]).T * scale),
        "wkT": np.ascontiguousarray(f32(inputs["wk"]).T),
        "wvT": np.ascontiguousarray(f32(inputs["wv"]).T),
        "woT": np.ascontiguousarray(f32(inputs["wo"]).T),
        "bq": f32(inputs["bq"]).reshape(C, 1) * scale,
        "bk": f32(inputs["bk"]).reshape(C, 1),
        "bv": f32(inputs["bv"]).reshape(C, 1),
        "bo": f32(inputs["bo"]).reshape(C, 1),
        "gmat": gmat,
        "gexp": np.ascontiguousarray(gmat.T),
        "ident": np.eye(C).astype(ml_dtypes.bfloat16),
    }
    in_maps = [
        {**shared, "x": np.ascontiguousarray(x[b].reshape(C, N))} for b in range(B)
    ]

    res = bass_utils.run_bass_kernel_spmd(nc, in_maps, core_ids=list(range(NCORES)))
    out = np.stack([res.results[b]["out"].reshape(C, H, W) for b in range(B)])
    return out.astype(np.float32)
